# revision 8
# baseline (speedup 1.0000x reference)
"""Trainium2 Bass kernel for nn_CascadedVMambaBlock — optimized v2.

Sharding: 8 cores; core c = (b, nh) with b = c//4, nh = c%4.
Each core processes sample b with state-dim slice n in [4nh, 4nh+4)
for ALL 4 scan directions k; per-stage AllReduces (bf16) over each
4-core b-group combine the n-partials of y.

Optimizations over v1:
- LN smalls batched across the 5 chunks on (5, CS) tiles via
  64-aligned accumulate-into-row stats matmuls; r/m broadcasts via
  bf16 select-row matmuls (1-pass PE instead of 4-pass fp32).
- LN1 applied to the 32-ch input (s_norm); one fused in_proj matmul
  produces xx and z; z silu'd straight out of PSUM.
- Depthwise conv in a 2-half layout (128 partitions, half the DVE
  columns); conv bias folded into the silu.
- u2 in bf16 (DVE 2x_1p); y accumulators and AllReduces in bf16.
- Ds*xs folded into k=0's PSUM accumulation via a diagonal matmul.
- k order (1,3,0,2): cm AllReduce after k=3 hides under the rm ks;
  rm AllReduce split into 3 pieces pipelined behind k=2's reverse
  sweep; out_norm consumes chunks in reverse order to match arrival.
"""
import numpy as np

HEAD, C_IN, C_H = 4, 128, 32
D, N, K, DT_RANK = 64, 16, 4, 2
B, H, W = 2, 48, 48
L = H * W            # 2304
CS = 512
CHUNKS = [(i * CS, min(CS, L - i * CS)) for i in range((L + CS - 1) // CS)]
NCH = len(CHUNKS)    # 5
SCS = 1024
SCHUNKS = [(i * SCS, min(SCS, L - i * SCS)) for i in range((L + SCS - 1) // SCS)]
EPS = 1e-5
NCORES = 8
HR = 24              # rows per conv half

_cache = {}


def _build(vs, cvm):
    import concourse.bass as bass
    import concourse.bacc as bacc
    import concourse.tile as tile
    import concourse.mybir as mybir
    from contextlib import ExitStack

    f32 = mybir.dt.float32
    f32r = mybir.dt.float32r
    bf16 = mybir.dt.bfloat16
    AF = mybir.ActivationFunctionType
    OP = mybir.AluOpType

    import concourse.hw_specs as hw_specs
    _orig_gat = hw_specs.get_activation_tables
    _KEEP = {"natural_log_exp_and_others", "silu_and_others"}

    def _patched_gat(arch):
        t = _orig_gat(arch)
        return {k: (v if k in _KEEP else set()) for k, v in t.items()}

    bacc.get_activation_tables = _patched_gat

    nc = bacc.Bacc("TRN2", target_bir_lowering=False, debug=False,
                   enable_asserts=True, num_devices=NCORES)

    def din(name, shape, dtype=f32):
        return nc.dram_tensor(name, shape, dtype, kind="ExternalInput").ap()

    x_shuf_d = din("x_shuf", (C_IN, L))
    lhsT_ip_d = din("lhsT_ip", (C_H, C_IN), f32r)     # [xx; z] weights
    bias_xx_d = din("bias_xx", (D, 1))
    bias_z_d = din("bias_z", (D, 1))
    w9_d = din("w9", (C_IN, 9))                       # conv taps dup 2 halves
    convb2_d = din("convb2", (C_IN, 1))
    sel128_d = din("sel128", (NCH, NCH * 128))        # 128-wide (final LN)
    selb_d = din("selb", (NCH, NCH * 64), bf16)       # same in bf16
    st32_d = din("st32", (C_H, NCH * 64), f32r)       # LN1 stats lhsT blocks
    st64_d = din("st64", (D, NCH * 64), bf16)         # out_norm stats lhsT
    st128_d = din("st128", (C_IN, NCH * 64), f32r)    # final stats lhsT
    lhsT_M2_d = din("lhsT_M2", (D, K, C_IN), bf16)
    dtb2_d = din("dtb2", (C_IN, K))
    A2_d = din("A2", (C_IN, K, 2))
    lhsT_B_d = din("lhsT_B", (D, K, 2, C_IN), bf16)
    lhsT_C_d = din("lhsT_C", (D, K, 2, C_IN), bf16)
    lhsT_ys_d = din("lhsT_ys", (C_IN, D), bf16)
    lhsT_ds_d = din("lhsT_ds", (D, D), bf16)          # diag(Ds_q)
    lhsT_op_d = din("lhsT_op", (D, C_H), f32r)
    gamma_d = din("gamma", (C_IN, 1))
    beta_d = din("beta", (C_IN, 1))

    out_d = nc.dram_tensor("out_cf", (C_IN, L), f32, kind="ExternalOutput").ap()

    with tile.TileContext(nc) as tc, ExitStack() as ctx:
        w_pool = ctx.enter_context(tc.tile_pool(name="weights", bufs=1))
        big = ctx.enter_context(tc.tile_pool(name="big", bufs=1))
        stg = ctx.enter_context(tc.tile_pool(name="stg", bufs=1))
        sml = ctx.enter_context(tc.tile_pool(name="sml", bufs=2))
        scn = ctx.enter_context(tc.tile_pool(name="scn", bufs=2))
        hpool = ctx.enter_context(tc.tile_pool(name="hpool", bufs=4))
        ps = ctx.enter_context(tc.tile_pool(name="ps", bufs=1, space="PSUM"))
        dram = ctx.enter_context(tc.tile_pool(name="dram", bufs=2, space="DRAM"))

        def wload(ap_d, shape, dtype=f32):
            t = w_pool.tile(list(shape), dtype, name=ap_d.tensor.name + "_sb")
            src = ap_d if ap_d.dtype == dtype else ap_d.bitcast(dtype)
            nc.sync.dma_start(t[:], src)
            return t

        x_shuf = wload(x_shuf_d, (C_IN, L), f32r)
        lhsT_ip = wload(lhsT_ip_d, (C_H, C_IN), f32r)
        bias_xx = wload(bias_xx_d, (D, 1))
        bias_z = wload(bias_z_d, (D, 1))
        w9 = wload(w9_d, (C_IN, 9))
        convb2 = wload(convb2_d, (C_IN, 1))
        sel128 = wload(sel128_d, (NCH, NCH * 128))
        selb = wload(selb_d, (NCH, NCH * 64), bf16)
        st32 = wload(st32_d, (C_H, NCH * 64), f32r)
        st64 = wload(st64_d, (D, NCH * 64), bf16)
        st128 = wload(st128_d, (C_IN, NCH * 64), f32r)
        lhsT_M2 = wload(lhsT_M2_d, (D, K, C_IN), bf16)
        dtb2 = wload(dtb2_d, (C_IN, K))
        A2 = wload(A2_d, (C_IN, K, 2))
        lhsT_B = wload(lhsT_B_d, (D, K, 2, C_IN), bf16)
        lhsT_C = wload(lhsT_C_d, (D, K, 2, C_IN), bf16)
        lhsT_ys = wload(lhsT_ys_d, (C_IN, D), bf16)
        lhsT_ds = wload(lhsT_ds_d, (D, D), bf16)
        lhsT_op = wload(lhsT_op_d, (D, C_H), f32r)
        gamma = wload(gamma_d, (C_IN, 1))
        beta = wload(beta_d, (C_IN, 1))

        # persistent big tensors
        xx_pad2 = big.tile([C_IN, 26 * 50], f32, tag="xxp")   # 2-half padded conv input
        nc.vector.memset(xx_pad2[:], 0.0)
        outs_cat = big.tile([C_IN, L], f32)
        xres = big.tile([C_IN, L], f32r)

        # batched LN smalls: (NCH, CS) psum mean/meansq rows -> rstd r5
        # and mean*rstd mr5, in `dt` (bf16 for head LNs, f32 for final).
        def ln_smalls5(ps_m5, ps_e5, tagp, dt):
            m2_5 = sml.tile([NCH, CS], f32, tag=tagp + "m2", name="m2_5")
            nc.scalar.square(m2_5[:], ps_m5[:])
            var5 = sml.tile([NCH, CS], f32, tag=tagp + "var", name="var5")
            nc.vector.scalar_tensor_tensor(var5[:], ps_e5[:], EPS,
                                           m2_5[:], OP.add, OP.subtract)
            lnv5 = sml.tile([NCH, CS], f32, tag=tagp + "m2", name="lnv5")
            nc.scalar.activation(lnv5[:], var5[:], AF.Ln)
            r5 = sml.tile([NCH, CS], dt, tag=tagp + "r5", name="r5",
                          bufs=1)
            nc.scalar.activation(r5[:], lnv5[:], AF.Exp, scale=-0.5)
            mr5 = sml.tile([NCH, CS], dt, tag=tagp + "mr5", name="mr5",
                           bufs=1)
            nc.vector.tensor_mul(mr5[:], ps_m5[:], r5[:])
            return r5, mr5

        # tiny dummy AllReduce: absorbs core-launch skew while the
        # front-end of head 0 runs, so head 0's real ARs don't eat it
        sync_in = dram.tile([1, 16], f32, tag="sync_in", name="sync_in")
        sync_out = dram.tile([1, 16], f32, tag="sync_out", name="sync_out")
        zrow = w_pool.tile([1, 16], f32, name="zrow")
        nc.vector.memset(zrow[:], 0.0)
        nc.sync.dma_start(sync_in[:], zrow[:])
        nc.gpsimd.collective_compute(
            "AllReduce", OP.add,
            replica_groups=[[0, 1, 2, 3], [4, 5, 6, 7]],
            ins=[sync_in[:].opt()], outs=[sync_out[:].opt()])

        prev_sb = None
        s_t = None
        for i in range(HEAD):
            # ---- stage input s (32, L) ----
            chunk_sb = sml.tile([C_H, L], f32r, tag="s_cs", name="chunk_sb")
            nc.sync.dma_start(chunk_sb[:], x_shuf[32 * i:32 * (i + 1), :])
            if i == 0:
                s_t = chunk_sb[:]
            else:
                s_new = sml.tile([C_H, L], f32r, tag="s_cs", name="s_new")
                nc.vector.tensor_add(s_new[:], prev_sb[:],
                                     chunk_sb[:].bitcast(f32))
                s_t = s_new[:]

            # ---- LN1 stats (batched smalls) ----
            ps_m5 = ps.tile([NCH, CS], f32, tag="sa", name="ps_m5")
            ps_e5 = ps.tile([NCH, CS], f32, tag="sd", name="ps_e5")
            sq_l = stg.tile([C_H, L], f32r, tag="sq_l", name="sq_l")
            for ci, (o, w) in enumerate(CHUNKS):
                nc.vector.tensor_mul(sq_l[:, o:o + w],
                                     s_t[:, o:o + w].bitcast(f32),
                                     s_t[:, o:o + w].bitcast(f32))
                nc.tensor.matmul(ps_m5[:, :w], st32[:, 64 * ci:64 * ci + NCH],
                                 s_t[:, o:o + w],
                                 start=(ci == 0), stop=(ci == NCH - 1))
                nc.tensor.matmul(ps_e5[:, :w], st32[:, 64 * ci:64 * ci + NCH],
                                 sq_l[:, o:o + w],
                                 start=(ci == 0), stop=(ci == NCH - 1))
            r5, mr5 = ln_smalls5(ps_m5, ps_e5, "sm", bf16)

            # ---- LN1 apply + fused in_proj; silu(z) from PSUM ----
            xzxx_sb = stg.tile([D, L], f32, tag="xzshare", name="xzxx_sb")
            sz_sb = stg.tile([D, L], bf16, tag="sz", name="sz_sb")
            for ci, (o, w) in enumerate(CHUNKS):
                ps_r32 = ps.tile([C_H, CS], f32, tag="sa", name="ps_r32")
                nc.tensor.matmul(ps_r32[:, :w],
                                 selb[:, 64 * ci:64 * ci + C_H], r5[:, :w],
                                 start=True, stop=True)
                ps_mr32 = ps.tile([C_H, CS], f32, tag="sd", name="ps_mr32")
                nc.tensor.matmul(ps_mr32[:, :w],
                                 selb[:, 64 * ci:64 * ci + C_H], mr5[:, :w],
                                 start=True, stop=True)
                sn_c = sml.tile([C_H, CS], f32r, tag="sn_c", name="sn_c")
                nc.vector.tensor_mul(sn_c[:, :w],
                                     s_t[:, o:o + w].bitcast(f32),
                                     ps_r32[:, :w])
                nc.vector.tensor_sub(sn_c[:, :w],
                                     sn_c[:, :w].bitcast(f32),
                                     ps_mr32[:, :w])
                ps_xz = ps.tile([C_IN, CS], f32, tag="pd", name="ps_xz", bufs=2)
                nc.tensor.matmul(ps_xz[:, :w], lhsT_ip[:], sn_c[:, :w],
                                 start=True, stop=True)
                nc.vector.tensor_scalar(xzxx_sb[:, o:o + w], ps_xz[0:D, :w],
                                        bias_xx[:], None, OP.add)
                nc.scalar.activation(sz_sb[:, o:o + w], ps_xz[D:C_IN, :w],
                                     AF.Silu, bias=bias_z[:])

            # assemble 2-half padded conv input
            xp0 = xx_pad2[0:D, :].rearrange("c (h w) -> c h w", h=26, w=50)
            xp1 = xx_pad2[D:C_IN, :].rearrange("c (h w) -> c h w", h=26, w=50)
            nc.sync.dma_start(
                xp0[:, 1:26, 1:49],
                xzxx_sb[:, 0:25 * 48].rearrange("c (h w) -> c h w", h=25, w=48))
            nc.sync.dma_start(
                xp1[:, 0:25, 1:49],
                xzxx_sb[:, 23 * 48:L].rearrange("c (h w) -> c h w", h=25, w=48))

            # ---- depthwise 3x3 conv on (128, 24*48) ----
            conv2 = stg.tile([C_IN, HR * 48], f32, tag="xzshare2", name="conv2")
            xpv = xx_pad2[:].rearrange("c (h w) -> c h w", h=26, w=50)
            cav = conv2[:].rearrange("c (h w) -> c h w", h=HR, w=48)
            first = True
            for dy in range(3):
                for dx in range(3):
                    tap = 3 * dy + dx
                    src_v = xpv[:, dy:dy + HR, dx:dx + 48]
                    if first:
                        nc.vector.tensor_scalar(cav, src_v, w9[:, tap:tap + 1],
                                                None, OP.mult)
                        first = False
                    else:
                        nc.vector.scalar_tensor_tensor(cav, src_v,
                                                       w9[:, tap:tap + 1],
                                                       cav, OP.mult, OP.add)

            # ---- silu(conv) + sequence orderings ----
            xs_sh = stg.tile([C_IN, HR * 48], bf16, tag="xssh", name="xs_sh")
            nc.scalar.activation(xs_sh[:], conv2[:], AF.Silu, bias=convb2[:])
            xs2_rm = stg.tile([C_IN, L], bf16, tag="xs2rm", name="xs2_rm")
            nc.sync.dma_start(xs2_rm[0:D, 0:HR * 48], xs_sh[0:D, :])
            nc.sync.dma_start(xs2_rm[0:D, HR * 48:L], xs_sh[D:C_IN, :])
            nc.sync.dma_start(xs2_rm[D:C_IN, :], xs2_rm[0:D, :])
            xs2_cm = stg.tile([C_IN, L], bf16, tag="xs2cm", name="xs2_cm")
            nc.vector.tensor_copy(
                xs2_cm[0:D, :].rearrange("c (w h) -> c w h", h=48, w=48),
                xs2_rm[0:D, :].rearrange("c (h w) -> c w h", h=48, w=48))
            nc.sync.dma_start(xs2_cm[D:C_IN, :], xs2_cm[0:D, :])

            # ---- scan core; k order: cm pair (1,3) then rm pair (0,2) ----
            y_mid = stg.tile([D, L], bf16, tag="ymid", name="y_mid")
            y_cm_acc = stg.tile([D, L], bf16, tag="ycm", name="y_cm_acc")
            y_cm_g = stg.tile([D, L], bf16, tag="ycmg", name="y_cm_g")
            ar_cm_in = dram.tile([D, L], bf16, tag="ar_cm_in", name="ar_cm_in")
            ar_cm_out = dram.tile([D, L], bf16, tag="ar_cm_out", name="ar_cm_out")
            for k in (1, 3, 0, 2):
                xs2 = xs2_rm if k in (0, 2) else xs2_cm
                rev = k >= 2

                h_prev = {0: None, 1: None}
                corder = list(range(len(SCHUNKS)))
                if rev:
                    corder = corder[::-1]
                for ci in corder:
                    o, w = SCHUNKS[ci]
                    subs = [(so, min(512, w - so)) for so in range(0, w, 512)]
                    dt2_c = scn.tile([C_IN, SCS], bf16, tag="dt2_c",
                                     name="dt2_c")
                    for so, sw in subs:
                        ps_dt = ps.tile([C_IN, 512], f32, tag="sa",
                                        name="ps_dt")
                        nc.tensor.matmul(ps_dt[:, :sw], lhsT_M2[:, k, :],
                                         xs2[0:D, o + so:o + so + sw],
                                         start=True, stop=True)
                        e_ch = scn.tile([C_IN, 512], f32, tag="e_ch",
                                        name="e_ch")
                        nc.scalar.activation(e_ch[:, :sw],
                                             ps_dt[:, :sw], AF.Exp,
                                             bias=dtb2[:, k:k + 1])
                        nc.scalar.activation(dt2_c[:, so:so + sw],
                                             e_ch[:, :sw],
                                             AF.Ln, bias=1.0)
                    u2_c = scn.tile([C_IN, SCS], bf16, tag="u2b", name="u2_c")
                    nc.vector.tensor_mul(u2_c[:, :w], dt2_c[:, :w],
                                         xs2[:, o:o + w])
                    ps_ys = {}
                    for gl in range(2):
                        dA = scn.tile([C_IN, SCS], f32, tag="dA", name="dA",
                                      bufs=1)
                        nc.scalar.activation(dA[:, :w], dt2_c[:, :w],
                                             AF.Exp, scale=A2[:, k, gl:gl + 1])
                        bB = scn.tile([C_IN, SCS], f32, tag="bB", name="bB")
                        ps_b = ps.tile([C_IN, SCS], f32, tag="sb",
                                       name="ps_b")
                        for so, sw in subs:
                            nc.tensor.matmul(ps_b[:, so:so + sw],
                                             lhsT_B[:, k, gl, :],
                                             xs2[0:D, o + so:o + so + sw],
                                             start=True, stop=True)
                        nc.vector.tensor_mul(bB[:, :w], u2_c[:, :w],
                                             ps_b[:, :w])
                        h_c = hpool.tile([C_IN, SCS], f32, tag="h", name="h_c")
                        hp = h_prev[gl]
                        if not rev:
                            init = 0.0 if hp is None else hp[0][:, hp[1] - 1:hp[1]]
                            nc.vector.tensor_tensor_scan(
                                h_c[:, :w], dA[:, :w], bB[:, :w], init,
                                OP.mult, OP.add)
                        else:
                            init = 0.0 if hp is None else hp[0][:, 0:1]
                            nc.vector.tensor_tensor_scan(
                                h_c[:, :w][:, ::-1], dA[:, :w][:, ::-1],
                                bB[:, :w][:, ::-1], init, OP.mult, OP.add)
                        h_prev[gl] = (h_c, w)
                        hC = scn.tile([C_IN, SCS], bf16, tag="hC", name="hC")
                        ps_c = ps.tile([C_IN, SCS], f32, tag="sc",
                                       name="ps_c")
                        for so, sw in subs:
                            nc.tensor.matmul(ps_c[:, so:so + sw],
                                             lhsT_C[:, k, gl, :],
                                             xs2[0:D, o + so:o + so + sw],
                                             start=True, stop=True)
                        nc.vector.tensor_mul(hC[:, :w], h_c[:, :w],
                                             ps_c[:, :w])
                        for so, sw in subs:
                            if gl == 0:
                                ps_ys[so] = ps.tile([D, 512], f32, tag="pd",
                                                    name="ps_y", bufs=2)
                            nc.tensor.matmul(ps_ys[so][:, :sw], lhsT_ys[:],
                                             hC[:, so:so + sw],
                                             start=(gl == 0),
                                             stop=(gl == 1 and k != 0),
                                             skip_group_check=True)
                    # k=0: accumulate Ds*xs into the same PSUM group
                    if k == 0:
                        for so, sw in subs:
                            go = o + so
                            nc.tensor.matmul(ps_ys[so][:, :sw], lhsT_ds[:],
                                             xs2[0:D, go:go + sw],
                                             start=False, stop=True,
                                             skip_group_check=True)
                    # drain sub-chunk y
                    for so, sw in subs:
                        go = o + so
                        if k == 1:
                            nc.scalar.copy(y_cm_acc[:, go:go + sw],
                                           ps_ys[so][:, :sw])
                        elif k == 3:
                            nc.vector.tensor_add(y_cm_acc[:, go:go + sw],
                                                 y_cm_acc[:, go:go + sw],
                                                 ps_ys[so][:, :sw])
                        elif k == 0:
                            nc.scalar.copy(y_mid[:, go:go + sw],
                                           ps_ys[so][:, :sw])
                        else:
                            nc.vector.tensor_add(y_mid[:, go:go + sw],
                                                 y_mid[:, go:go + sw],
                                                 ps_ys[so][:, :sw])
                    if k == 2:
                        # rm AR piece per reverse chunk (k=0 already added)
                        alo, ahi = o, o + w
                        aw = ahi - alo
                        ari = dram.tile([D, aw], bf16, tag=f"ar_rm_in{ci}",
                                        name="ari", bufs=2)
                        aro = dram.tile([D, aw], bf16, tag=f"ar_rm_out{ci}",
                                        name="aro", bufs=2)
                        nc.sync.dma_start(ari[:], y_mid[:, alo:ahi])
                        nc.gpsimd.collective_compute(
                            "AllReduce", OP.add,
                            replica_groups=[[0, 1, 2, 3], [4, 5, 6, 7]],
                            ins=[ari[:].opt()], outs=[aro[:].opt()])
                        nc.sync.dma_start(y_mid[:, alo:ahi], aro[:])
                if k == 3:
                    # cm pair complete: full AR + gather, hidden under rm
                    nc.sync.dma_start(ar_cm_in[:], y_cm_acc[:])
                    nc.gpsimd.collective_compute(
                        "AllReduce", OP.add,
                        replica_groups=[[0, 1, 2, 3], [4, 5, 6, 7]],
                        ins=[ar_cm_in[:].opt()], outs=[ar_cm_out[:].opt()])
                    nc.sync.dma_start(y_cm_acc[:], ar_cm_out[:])
                    nc.vector.tensor_copy(
                        y_cm_g[:].rearrange("c (h w) -> c h w", h=48, w=48),
                        y_cm_acc[:].rearrange("c (w h) -> c h w", h=48, w=48))

            # ---- out_norm (batched smalls) + *silu(z) + out_proj + res ----
            y_h = stg.tile([D, L], bf16, tag="yh", name="y_h")

            ps_m5o = ps.tile([NCH, CS], f32, tag="sa", name="ps_m5o")
            ps_e5o = ps.tile([NCH, CS], f32, tag="sd", name="ps_e5o")
            nco = NCH - 1
            for j, (o, w) in enumerate(reversed(CHUNKS)):
                ci = nco - j
                nc.vector.tensor_add(y_h[:, o:o + w], y_mid[:, o:o + w],
                                     y_cm_g[:, o:o + w])
                ysq_c = sml.tile([D, CS], bf16, tag="ysq_c", name="ysq_c")
                nc.scalar.square(ysq_c[:, :w], y_h[:, o:o + w])
                nc.tensor.matmul(ps_m5o[:, :w], st64[:, 64 * ci:64 * ci + NCH],
                                 y_h[:, o:o + w],
                                 start=(j == 0), stop=(j == nco))
                nc.tensor.matmul(ps_e5o[:, :w], st64[:, 64 * ci:64 * ci + NCH],
                                 ysq_c[:, :w],
                                 start=(j == 0), stop=(j == nco))
            r5o, mr5o = ln_smalls5(ps_m5o, ps_e5o, "sm", bf16)

            prev_new = stg.tile([C_H, L], f32, tag="prev", name="prev_new")
            for j, (o, w) in enumerate(reversed(CHUNKS)):
                ci = nco - j
                ps_ra = ps.tile([D, CS], f32, tag="sa", name="ps_ra")
                nc.tensor.matmul(ps_ra[:, :w], selb[:, 64 * ci:64 * ci + D],
                                 r5o[:, :w], start=True, stop=True)
                ps_rb = ps.tile([D, CS], f32, tag="sd", name="ps_rb")
                nc.tensor.matmul(ps_rb[:, :w], selb[:, 64 * ci:64 * ci + D],
                                 mr5o[:, :w], start=True, stop=True)
                t1_c = sml.tile([D, CS], f32, tag="t1_c", name="t1_c")
                nc.vector.tensor_mul(t1_c[:, :w], y_h[:, o:o + w],
                                     ps_ra[:, :w])
                nc.vector.tensor_sub(t1_c[:, :w], t1_c[:, :w], ps_rb[:, :w])
                y2_c = sml.tile([D, CS], f32r, tag="y2_c", name="y2_c")
                nc.vector.tensor_mul(y2_c[:, :w], t1_c[:, :w],
                                     sz_sb[:, o:o + w])
                ps_op = ps.tile([C_H, CS], f32, tag="pd", name="ps_op", bufs=2)
                nc.tensor.matmul(ps_op[:, :w], lhsT_op[:],
                                 y2_c[:, :w], start=True, stop=True)
                nc.vector.scalar_tensor_tensor(
                    prev_new[:, o:o + w], s_t[:, o:o + w].bitcast(f32),
                    1.0 + vs, ps_op[:, :w], OP.mult, OP.add)
            prev_sb = prev_new
            nc.sync.dma_start(outs_cat[32 * i:32 * (i + 1), :], prev_new[:])

        # ---- final: x_res = cvm*x_shuf + outs_cat; LN over 128 ch ----
        nc.vector.scalar_tensor_tensor(xres[:],
                                       x_shuf[:].bitcast(f32), cvm,
                                       outs_cat[:], OP.mult, OP.add)
        out_sb = big.tile([C_IN, L], f32, tag="outs_cat", name="out_sb")
        xsq_l = big.tile([C_IN, L], f32r, tag="xxp2", name="xsq_l")
        ps_m5f = ps.tile([NCH, CS], f32, tag="sa", name="ps_m5f")
        ps_e5f = ps.tile([NCH, CS], f32, tag="sd", name="ps_e5f")
        for ci, (o, w) in enumerate(CHUNKS):
            nc.vector.tensor_mul(xsq_l[:, o:o + w],
                                 xres[:, o:o + w].bitcast(f32),
                                 xres[:, o:o + w].bitcast(f32))
            nc.tensor.matmul(ps_m5f[:, :w], st128[:, 64 * ci:64 * ci + NCH],
                             xres[:, o:o + w],
                             start=(ci == 0), stop=(ci == NCH - 1))
            nc.tensor.matmul(ps_e5f[:, :w], st128[:, 64 * ci:64 * ci + NCH],
                             xsq_l[:, o:o + w],
                             start=(ci == 0), stop=(ci == NCH - 1))
        r5f, mr5f = ln_smalls5(ps_m5f, ps_e5f, "sm", f32)
        for ci, (o, w) in enumerate(CHUNKS):
            ps_ra = ps.tile([C_IN, CS], f32, tag="sa", name="ps_ra3")
            nc.tensor.matmul(ps_ra[:, :w], sel128[:, 128 * ci:128 * ci + 128],
                             r5f[:, :w], start=True, stop=True)
            ps_rb = ps.tile([C_IN, CS], f32, tag="sd", name="ps_rb3")
            nc.tensor.matmul(ps_rb[:, :w], sel128[:, 128 * ci:128 * ci + 128],
                             mr5f[:, :w], start=True, stop=True)
            nc.vector.tensor_mul(out_sb[:, o:o + w],
                                 xres[:, o:o + w].bitcast(f32), ps_ra[:, :w])
            nc.vector.tensor_sub(out_sb[:, o:o + w], out_sb[:, o:o + w],
                                 ps_rb[:, :w])
            nc.vector.tensor_scalar(out_sb[:, o:o + w], out_sb[:, o:o + w],
                                    gamma[:], beta[:], OP.mult, OP.add)
            nc.sync.dma_start(out_d[:, o:o + w], out_sb[:, o:o + w])

    nc.compile()
    return nc


def _host_prep(inputs):
    import ml_dtypes
    bf16 = ml_dtypes.bfloat16
    x = np.asarray(inputs["x"], np.float32)
    ln1_w = np.asarray(inputs["ln1_w"], np.float32)
    ln1_b = np.asarray(inputs["ln1_b"], np.float32)
    in_proj_w = np.asarray(inputs["in_proj_w"], np.float32)
    conv_w = np.asarray(inputs["conv_w"], np.float32)
    conv_b = np.asarray(inputs["conv_b"], np.float32)
    x_proj_w = np.asarray(inputs["x_proj_w"], np.float32)
    dt_proj_w = np.asarray(inputs["dt_proj_w"], np.float32)
    dt_proj_b = np.asarray(inputs["dt_proj_b"], np.float32)
    A_logs = np.asarray(inputs["A_logs"], np.float32)
    Ds = np.asarray(inputs["Ds"], np.float32)
    out_norm_w = np.asarray(inputs["out_norm_w"], np.float32)
    out_norm_b = np.asarray(inputs["out_norm_b"], np.float32)
    out_proj_w = np.asarray(inputs["out_proj_w"], np.float32)
    final_ln_w = np.asarray(inputs["final_ln_w"], np.float32)
    final_ln_b = np.asarray(inputs["final_ln_b"], np.float32)
    assert not np.any(out_norm_b), "out_norm_b must be zero (folded)"

    # LN1 is applied to the input now: xx|z = W_ip @ (norm(s)*g + b);
    # gamma folds into W_ip, ln1_b goes through in_proj as a bias.
    W_ip = (in_proj_w * ln1_w[None, :]).astype(np.float32)        # (128, 32)
    bias_ip = (in_proj_w @ ln1_b).astype(np.float32)              # (128,)
    lhsT_ip = np.ascontiguousarray(W_ip.T)                        # (32, 128)
    w9_64 = np.ascontiguousarray(
        conv_w[:, :, 0, :].transpose(2, 0, 1).reshape(D, 9))      # (64, 9)
    w9 = np.concatenate([w9_64, w9_64], 0)                        # (128, 9)
    convb2 = np.concatenate([conv_b, conv_b]).reshape(C_IN, 1)
    A = -np.exp(A_logs)                                           # (K, 64, 16)
    Ds_q = (Ds.sum(0) / 4.0).astype(np.float32)                   # (64,)
    lhsT_ds = np.zeros((D, D), bf16)
    for d in range(D):
        lhsT_ds[d, d] = Ds_q[d]
    W_op = (out_proj_w * out_norm_w[None, :]).astype(np.float32)  # (32, 64)
    lhsT_op = np.ascontiguousarray(W_op.T)                        # (64, 32)

    M = np.einsum("kdr,krc->kdc", dt_proj_w, x_proj_w[:, :DT_RANK, :])
    lhsT_M2 = np.zeros((D, K, C_IN), bf16)
    for k in range(K):
        lhsT_M2[:, k, 0:D] = M[k].T
        lhsT_M2[:, k, D:C_IN] = M[k].T
    dtb2 = np.zeros((C_IN, K), np.float32)
    dtb2[0:D] = dt_proj_b.T
    dtb2[D:C_IN] = dt_proj_b.T

    # select-row / stats lhsT blocks (64-aligned per chunk)
    sel64 = np.zeros((NCH, NCH * 64), np.float32)
    sel128 = np.zeros((NCH, NCH * 128), np.float32)
    for ci in range(NCH):
        sel64[ci, 64 * ci:64 * ci + 64] = 1.0
        sel128[ci, 128 * ci:128 * ci + 128] = 1.0
    st32 = np.zeros((C_H, NCH * 64), np.float32)
    st64 = np.zeros((D, NCH * 64), np.float32)
    st128 = np.zeros((C_IN, NCH * 64), np.float32)
    for ci in range(NCH):
        st32[:, 64 * ci + ci] = 1.0 / C_H
        st64[:, 64 * ci + ci] = 1.0 / D
        st128[:, 64 * ci + ci] = 1.0 / C_IN
    lhsT_ys = np.zeros((C_IN, D), bf16)
    for p in range(C_IN):
        lhsT_ys[p, p % D] = 1.0

    common = {
        "lhsT_ip": lhsT_ip,
        "bias_xx": bias_ip[0:D].reshape(D, 1),
        "bias_z": bias_ip[D:C_IN].reshape(D, 1),
        "w9": w9, "convb2": convb2,
        "sel128": sel128, "selb": sel64.astype(bf16),
        "st32": st32, "st64": st64.astype(bf16), "st128": st128,
        "lhsT_M2": lhsT_M2, "dtb2": dtb2,
        "lhsT_ys": lhsT_ys, "lhsT_ds": lhsT_ds,
        "lhsT_op": lhsT_op,
        "gamma": final_ln_w.reshape(C_IN, 1),
        "beta": final_ln_b.reshape(C_IN, 1),
    }

    g = HEAD
    per_b = []
    for b in range(B):
        xs = x[b].reshape(H, W, g, C_IN // g).transpose(0, 1, 3, 2).reshape(L, C_IN)
        per_b.append(np.ascontiguousarray(xs.T))

    in_maps = []
    for c in range(NCORES):
        b, nh = c // 4, c % 4
        A2 = np.zeros((C_IN, K, 2), np.float32)
        lhsT_B = np.zeros((D, K, 2, C_IN), bf16)
        lhsT_C = np.zeros((D, K, 2, C_IN), bf16)
        for k in range(K):
            for gl in range(2):
                for half in range(2):
                    n = 4 * nh + 2 * gl + half
                    rows = slice(64 * half, 64 * half + 64)
                    A2[rows, k, gl] = A[k, :, n]
                    lhsT_B[:, k, gl, rows] = x_proj_w[k, DT_RANK + n, :][:, None]
                    lhsT_C[:, k, gl, rows] = x_proj_w[k, DT_RANK + N + n, :][:, None]
        in_maps.append(dict(common, x_shuf=per_b[b],
                            A2=A2, lhsT_B=lhsT_B, lhsT_C=lhsT_C))
    vs = float(np.asarray(inputs["vss_skip"]).ravel()[0])
    cvm = float(np.asarray(inputs["cvm_skip"]).ravel()[0])
    return in_maps, vs, cvm


def kernel(**inputs) -> np.ndarray:
    from concourse.bass_utils import run_bass_kernel_spmd

    in_maps, vs, cvm = _host_prep(inputs)
    key = (vs, cvm)
    if key not in _cache:
        _cache[key] = _build(vs, cvm)
    nc = _cache[key]
    res = run_bass_kernel_spmd(nc, in_maps, core_ids=list(range(NCORES)))
    out = np.zeros((B, H, W, C_IN), np.float32)
    for b in range(B):
        out_cf = res.results[4 * b]["out_cf"]
        out[b] = out_cf.T.reshape(H, W, C_IN)
    return out


# revision 9
# speedup vs baseline: 1.0055x; 1.0055x over previous
"""Trainium2 Bass kernel for nn_CascadedVMambaBlock — optimized v2.

Sharding: 8 cores; core c = (b, nh) with b = c//4, nh = c%4.
Each core processes sample b with state-dim slice n in [4nh, 4nh+4)
for ALL 4 scan directions k; per-stage AllReduces (bf16) over each
4-core b-group combine the n-partials of y.

Optimizations over v1:
- LN smalls batched across the 5 chunks on (5, CS) tiles via
  64-aligned accumulate-into-row stats matmuls; r/m broadcasts via
  bf16 select-row matmuls (1-pass PE instead of 4-pass fp32).
- LN1 applied to the 32-ch input (s_norm); one fused in_proj matmul
  produces xx and z; z silu'd straight out of PSUM.
- Depthwise conv in a 2-half layout (128 partitions, half the DVE
  columns); conv bias folded into the silu.
- u2 in bf16 (DVE 2x_1p); y accumulators and AllReduces in bf16.
- Ds*xs folded into k=0's PSUM accumulation via a diagonal matmul.
- k order (1,3,0,2): cm AllReduce after k=3 hides under the rm ks;
  rm AllReduce split into 3 pieces pipelined behind k=2's reverse
  sweep; out_norm consumes chunks in reverse order to match arrival.
"""
import numpy as np

HEAD, C_IN, C_H = 4, 128, 32
D, N, K, DT_RANK = 64, 16, 4, 2
B, H, W = 2, 48, 48
L = H * W            # 2304
CS = 512
CHUNKS = [(i * CS, min(CS, L - i * CS)) for i in range((L + CS - 1) // CS)]
NCH = len(CHUNKS)    # 5
SCS = 1024
SCHUNKS = [(i * SCS, min(SCS, L - i * SCS)) for i in range((L + SCS - 1) // SCS)]
EPS = 1e-5
NCORES = 8
HR = 24              # rows per conv half

_cache = {}


def _build(vs, cvm):
    import concourse.bass as bass
    import concourse.bacc as bacc
    import concourse.tile as tile
    import concourse.mybir as mybir
    from contextlib import ExitStack

    f32 = mybir.dt.float32
    f32r = mybir.dt.float32r
    bf16 = mybir.dt.bfloat16
    AF = mybir.ActivationFunctionType
    OP = mybir.AluOpType

    import concourse.hw_specs as hw_specs
    _orig_gat = hw_specs.get_activation_tables
    _KEEP = {"natural_log_exp_and_others", "silu_and_others"}

    def _patched_gat(arch):
        t = _orig_gat(arch)
        return {k: (v if k in _KEEP else set()) for k, v in t.items()}

    bacc.get_activation_tables = _patched_gat

    nc = bacc.Bacc("TRN2", target_bir_lowering=False, debug=False,
                   enable_asserts=True, num_devices=NCORES)

    def din(name, shape, dtype=f32):
        return nc.dram_tensor(name, shape, dtype, kind="ExternalInput").ap()

    x_shuf_d = din("x_shuf", (C_IN, L))
    lhsT_ip_d = din("lhsT_ip", (C_H, C_IN), f32r)     # [xx; z] weights
    bias_xx_d = din("bias_xx", (D, 1))
    bias_z_d = din("bias_z", (D, 1))
    w9_d = din("w9", (C_IN, 9))                       # conv taps dup 2 halves
    convb2_d = din("convb2", (C_IN, 1))
    sel128_d = din("sel128", (NCH, NCH * 128))        # 128-wide (final LN)
    selb_d = din("selb", (NCH, NCH * 64), bf16)       # same in bf16
    st32_d = din("st32", (C_H, NCH * 64), f32r)       # LN1 stats lhsT blocks
    st64_d = din("st64", (D, NCH * 64), bf16)         # out_norm stats lhsT
    st128_d = din("st128", (C_IN, NCH * 64), f32r)    # final stats lhsT
    lhsT_M2_d = din("lhsT_M2", (D, K, C_IN), bf16)
    dtb2_d = din("dtb2", (C_IN, K))
    A2_d = din("A2", (C_IN, K, 2))
    lhsT_B_d = din("lhsT_B", (D, K, 2, C_IN), bf16)
    lhsT_C_d = din("lhsT_C", (D, K, 2, C_IN), bf16)
    lhsT_ys_d = din("lhsT_ys", (C_IN, D), bf16)
    lhsT_ds_d = din("lhsT_ds", (D, D), bf16)          # diag(Ds_q)
    lhsT_op_d = din("lhsT_op", (D, C_H), f32r)
    gamma_d = din("gamma", (C_IN, 1))
    beta_d = din("beta", (C_IN, 1))

    out_d = nc.dram_tensor("out_cf", (C_IN, L), f32, kind="ExternalOutput").ap()

    with tile.TileContext(nc) as tc, ExitStack() as ctx:
        w_pool = ctx.enter_context(tc.tile_pool(name="weights", bufs=1))
        big = ctx.enter_context(tc.tile_pool(name="big", bufs=1))
        stg = ctx.enter_context(tc.tile_pool(name="stg", bufs=1))
        sml = ctx.enter_context(tc.tile_pool(name="sml", bufs=2))
        scn = ctx.enter_context(tc.tile_pool(name="scn", bufs=2))
        hpool = ctx.enter_context(tc.tile_pool(name="hpool", bufs=4))
        ps = ctx.enter_context(tc.tile_pool(name="ps", bufs=1, space="PSUM"))
        dram = ctx.enter_context(tc.tile_pool(name="dram", bufs=2, space="DRAM"))

        def wload(ap_d, shape, dtype=f32):
            t = w_pool.tile(list(shape), dtype, name=ap_d.tensor.name + "_sb")
            src = ap_d if ap_d.dtype == dtype else ap_d.bitcast(dtype)
            nc.sync.dma_start(t[:], src)
            return t

        x_shuf = wload(x_shuf_d, (C_IN, L), f32r)
        lhsT_ip = wload(lhsT_ip_d, (C_H, C_IN), f32r)
        bias_xx = wload(bias_xx_d, (D, 1))
        bias_z = wload(bias_z_d, (D, 1))
        w9 = wload(w9_d, (C_IN, 9))
        convb2 = wload(convb2_d, (C_IN, 1))
        sel128 = wload(sel128_d, (NCH, NCH * 128))
        selb = wload(selb_d, (NCH, NCH * 64), bf16)
        st32 = wload(st32_d, (C_H, NCH * 64), f32r)
        st64 = wload(st64_d, (D, NCH * 64), bf16)
        st128 = wload(st128_d, (C_IN, NCH * 64), f32r)
        lhsT_M2 = wload(lhsT_M2_d, (D, K, C_IN), bf16)
        dtb2 = wload(dtb2_d, (C_IN, K))
        A2 = wload(A2_d, (C_IN, K, 2))
        lhsT_B = wload(lhsT_B_d, (D, K, 2, C_IN), bf16)
        lhsT_C = wload(lhsT_C_d, (D, K, 2, C_IN), bf16)
        lhsT_ys = wload(lhsT_ys_d, (C_IN, D), bf16)
        lhsT_ds = wload(lhsT_ds_d, (D, D), bf16)
        lhsT_op = wload(lhsT_op_d, (D, C_H), f32r)
        gamma = wload(gamma_d, (C_IN, 1))
        beta = wload(beta_d, (C_IN, 1))

        # persistent big tensors
        xx_pad2 = big.tile([C_IN, 26 * 50], f32, tag="xxp")   # 2-half padded conv input
        nc.vector.memset(xx_pad2[:], 0.0)
        outs_cat = big.tile([C_IN, L], f32)
        xres = big.tile([C_IN, L], f32r)

        # batched LN smalls: (NCH, CS) psum mean/meansq rows -> rstd r5
        # and mean*rstd mr5, in `dt` (bf16 for head LNs, f32 for final).
        def ln_smalls5(ps_m5, ps_e5, tagp, dt):
            m2_5 = sml.tile([NCH, CS], f32, tag=tagp + "m2", name="m2_5")
            nc.scalar.square(m2_5[:], ps_m5[:])
            var5 = sml.tile([NCH, CS], f32, tag=tagp + "var", name="var5")
            nc.vector.scalar_tensor_tensor(var5[:], ps_e5[:], EPS,
                                           m2_5[:], OP.add, OP.subtract)
            lnv5 = sml.tile([NCH, CS], f32, tag=tagp + "m2", name="lnv5")
            nc.scalar.activation(lnv5[:], var5[:], AF.Ln)
            r5 = sml.tile([NCH, CS], dt, tag=tagp + "r5", name="r5",
                          bufs=1)
            nc.scalar.activation(r5[:], lnv5[:], AF.Exp, scale=-0.5)
            mr5 = sml.tile([NCH, CS], dt, tag=tagp + "mr5", name="mr5",
                           bufs=1)
            nc.vector.tensor_mul(mr5[:], ps_m5[:], r5[:])
            return r5, mr5

        # tiny dummy AllReduce: absorbs core-launch skew while the
        # front-end of head 0 runs, so head 0's real ARs don't eat it
        sync_in = dram.tile([1, 16], f32, tag="sync_in", name="sync_in")
        sync_out = dram.tile([1, 16], f32, tag="sync_out", name="sync_out")
        zrow = w_pool.tile([1, 16], f32, name="zrow")
        nc.vector.memset(zrow[:], 0.0)
        nc.sync.dma_start(sync_in[:], zrow[:])
        nc.gpsimd.collective_compute(
            "AllReduce", OP.add,
            replica_groups=[[0, 1, 2, 3], [4, 5, 6, 7]],
            ins=[sync_in[:].opt()], outs=[sync_out[:].opt()])

        prev_sb = None
        s_t = None
        for i in range(HEAD):
            # ---- stage input s (32, L) ----
            chunk_sb = sml.tile([C_H, L], f32r, tag="s_cs", name="chunk_sb")
            nc.sync.dma_start(chunk_sb[:], x_shuf[32 * i:32 * (i + 1), :])
            if i == 0:
                s_t = chunk_sb[:]
            else:
                s_new = sml.tile([C_H, L], f32r, tag="s_cs", name="s_new")
                nc.vector.tensor_add(s_new[:], prev_sb[:],
                                     chunk_sb[:].bitcast(f32))
                s_t = s_new[:]

            # ---- LN1 stats (batched smalls) ----
            ps_m5 = ps.tile([NCH, CS], f32, tag="sa", name="ps_m5")
            ps_e5 = ps.tile([NCH, CS], f32, tag="sd", name="ps_e5")
            sq_l = stg.tile([C_H, L], f32r, tag="sq_l", name="sq_l")
            for ci, (o, w) in enumerate(CHUNKS):
                nc.vector.tensor_mul(sq_l[:, o:o + w],
                                     s_t[:, o:o + w].bitcast(f32),
                                     s_t[:, o:o + w].bitcast(f32))
                nc.tensor.matmul(ps_m5[:, :w], st32[:, 64 * ci:64 * ci + NCH],
                                 s_t[:, o:o + w],
                                 start=(ci == 0), stop=(ci == NCH - 1))
                nc.tensor.matmul(ps_e5[:, :w], st32[:, 64 * ci:64 * ci + NCH],
                                 sq_l[:, o:o + w],
                                 start=(ci == 0), stop=(ci == NCH - 1))
            r5, mr5 = ln_smalls5(ps_m5, ps_e5, "sm", bf16)

            # ---- LN1 apply + fused in_proj; silu(z) from PSUM ----
            xzxx_sb = stg.tile([D, L], f32, tag="xzshare", name="xzxx_sb")
            sz_sb = stg.tile([D, L], bf16, tag="sz", name="sz_sb")
            for ci, (o, w) in enumerate(CHUNKS):
                ps_r32 = ps.tile([C_H, CS], f32, tag="sa", name="ps_r32")
                nc.tensor.matmul(ps_r32[:, :w],
                                 selb[:, 64 * ci:64 * ci + C_H], r5[:, :w],
                                 start=True, stop=True)
                ps_mr32 = ps.tile([C_H, CS], f32, tag="sd", name="ps_mr32")
                nc.tensor.matmul(ps_mr32[:, :w],
                                 selb[:, 64 * ci:64 * ci + C_H], mr5[:, :w],
                                 start=True, stop=True)
                sn_c = sml.tile([C_H, CS], f32r, tag="sn_c", name="sn_c")
                nc.vector.tensor_mul(sn_c[:, :w],
                                     s_t[:, o:o + w].bitcast(f32),
                                     ps_r32[:, :w])
                nc.vector.tensor_sub(sn_c[:, :w],
                                     sn_c[:, :w].bitcast(f32),
                                     ps_mr32[:, :w])
                ps_xz = ps.tile([C_IN, CS], f32, tag="pd", name="ps_xz", bufs=2)
                nc.tensor.matmul(ps_xz[:, :w], lhsT_ip[:], sn_c[:, :w],
                                 start=True, stop=True)
                nc.vector.tensor_scalar(xzxx_sb[:, o:o + w], ps_xz[0:D, :w],
                                        bias_xx[:], None, OP.add)
                nc.scalar.activation(sz_sb[:, o:o + w], ps_xz[D:C_IN, :w],
                                     AF.Silu, bias=bias_z[:])

            # assemble 2-half padded conv input
            xp0 = xx_pad2[0:D, :].rearrange("c (h w) -> c h w", h=26, w=50)
            xp1 = xx_pad2[D:C_IN, :].rearrange("c (h w) -> c h w", h=26, w=50)
            nc.sync.dma_start(
                xp0[:, 1:26, 1:49],
                xzxx_sb[:, 0:25 * 48].rearrange("c (h w) -> c h w", h=25, w=48))
            nc.sync.dma_start(
                xp1[:, 0:25, 1:49],
                xzxx_sb[:, 23 * 48:L].rearrange("c (h w) -> c h w", h=25, w=48))

            # ---- depthwise 3x3 conv on (128, 24*48) ----
            conv2 = stg.tile([C_IN, HR * 48], f32, tag="xzshare2", name="conv2")
            xpv = xx_pad2[:].rearrange("c (h w) -> c h w", h=26, w=50)
            cav = conv2[:].rearrange("c (h w) -> c h w", h=HR, w=48)
            first = True
            for dy in range(3):
                for dx in range(3):
                    tap = 3 * dy + dx
                    src_v = xpv[:, dy:dy + HR, dx:dx + 48]
                    if first:
                        nc.vector.tensor_scalar(cav, src_v, w9[:, tap:tap + 1],
                                                None, OP.mult)
                        first = False
                    else:
                        nc.vector.scalar_tensor_tensor(cav, src_v,
                                                       w9[:, tap:tap + 1],
                                                       cav, OP.mult, OP.add)

            # ---- silu(conv) + sequence orderings ----
            xs_sh = stg.tile([C_IN, HR * 48], bf16, tag="xssh", name="xs_sh")
            nc.scalar.activation(xs_sh[:], conv2[:], AF.Silu, bias=convb2[:])
            xs2_rm = stg.tile([C_IN, L], bf16, tag="xs2rm", name="xs2_rm")
            nc.sync.dma_start(xs2_rm[0:D, 0:HR * 48], xs_sh[0:D, :])
            nc.sync.dma_start(xs2_rm[0:D, HR * 48:L], xs_sh[D:C_IN, :])
            nc.sync.dma_start(xs2_rm[D:C_IN, :], xs2_rm[0:D, :])
            xs2_cm = stg.tile([C_IN, L], bf16, tag="xs2cm", name="xs2_cm")
            nc.vector.tensor_copy(
                xs2_cm[0:D, :].rearrange("c (w h) -> c w h", h=48, w=48),
                xs2_rm[0:D, :].rearrange("c (h w) -> c w h", h=48, w=48))
            nc.sync.dma_start(xs2_cm[D:C_IN, :], xs2_cm[0:D, :])

            # ---- scan core; k order: cm pair (1,3) then rm pair (0,2) ----
            y_mid = stg.tile([D, L], bf16, tag="ymid", name="y_mid")
            y_cm_acc = stg.tile([D, L], bf16, tag="ycm", name="y_cm_acc")
            y_cm_g = stg.tile([D, L], bf16, tag="ycmg", name="y_cm_g")
            for k in (2, 0, 1, 3):
                xs2 = xs2_rm if k in (0, 2) else xs2_cm
                rev = k >= 2

                h_prev = {0: None, 1: None}
                corder = list(range(len(SCHUNKS)))
                if rev:
                    corder = corder[::-1]
                for ci in corder:
                    o, w = SCHUNKS[ci]
                    subs = [(so, min(512, w - so)) for so in range(0, w, 512)]
                    dt2_c = scn.tile([C_IN, SCS], bf16, tag="dt2_c",
                                     name="dt2_c")
                    for so, sw in subs:
                        ps_dt = ps.tile([C_IN, 512], f32, tag="sa",
                                        name="ps_dt")
                        nc.tensor.matmul(ps_dt[:, :sw], lhsT_M2[:, k, :],
                                         xs2[0:D, o + so:o + so + sw],
                                         start=True, stop=True)
                        e_ch = scn.tile([C_IN, 512], f32, tag="e_ch",
                                        name="e_ch")
                        nc.scalar.activation(e_ch[:, :sw],
                                             ps_dt[:, :sw], AF.Exp,
                                             bias=dtb2[:, k:k + 1])
                        nc.scalar.activation(dt2_c[:, so:so + sw],
                                             e_ch[:, :sw],
                                             AF.Ln, bias=1.0)
                    u2_c = scn.tile([C_IN, SCS], bf16, tag="u2b", name="u2_c")
                    nc.vector.tensor_mul(u2_c[:, :w], dt2_c[:, :w],
                                         xs2[:, o:o + w])
                    ps_ys = {}
                    for gl in range(2):
                        dA = scn.tile([C_IN, SCS], f32, tag="dA", name="dA",
                                      bufs=1)
                        nc.scalar.activation(dA[:, :w], dt2_c[:, :w],
                                             AF.Exp, scale=A2[:, k, gl:gl + 1])
                        bB = scn.tile([C_IN, SCS], f32, tag="bB", name="bB")
                        ps_b = ps.tile([C_IN, SCS], f32, tag="sb",
                                       name="ps_b")
                        for so, sw in subs:
                            nc.tensor.matmul(ps_b[:, so:so + sw],
                                             lhsT_B[:, k, gl, :],
                                             xs2[0:D, o + so:o + so + sw],
                                             start=True, stop=True)
                        nc.vector.tensor_mul(bB[:, :w], u2_c[:, :w],
                                             ps_b[:, :w])
                        h_c = hpool.tile([C_IN, SCS], f32, tag="h", name="h_c")
                        hp = h_prev[gl]
                        if not rev:
                            init = 0.0 if hp is None else hp[0][:, hp[1] - 1:hp[1]]
                            nc.vector.tensor_tensor_scan(
                                h_c[:, :w], dA[:, :w], bB[:, :w], init,
                                OP.mult, OP.add)
                        else:
                            init = 0.0 if hp is None else hp[0][:, 0:1]
                            nc.vector.tensor_tensor_scan(
                                h_c[:, :w][:, ::-1], dA[:, :w][:, ::-1],
                                bB[:, :w][:, ::-1], init, OP.mult, OP.add)
                        h_prev[gl] = (h_c, w)
                        hC = scn.tile([C_IN, SCS], bf16, tag="hC", name="hC")
                        ps_c = ps.tile([C_IN, SCS], f32, tag="sc",
                                       name="ps_c")
                        for so, sw in subs:
                            nc.tensor.matmul(ps_c[:, so:so + sw],
                                             lhsT_C[:, k, gl, :],
                                             xs2[0:D, o + so:o + so + sw],
                                             start=True, stop=True)
                        nc.vector.tensor_mul(hC[:, :w], h_c[:, :w],
                                             ps_c[:, :w])
                        for so, sw in subs:
                            if gl == 0:
                                ps_ys[so] = ps.tile([D, 512], f32, tag="pd",
                                                    name="ps_y", bufs=2)
                            nc.tensor.matmul(ps_ys[so][:, :sw], lhsT_ys[:],
                                             hC[:, so:so + sw],
                                             start=(gl == 0),
                                             stop=(gl == 1 and k != 2),
                                             skip_group_check=True)
                    # k=2: accumulate Ds*xs into the same PSUM group
                    if k == 2:
                        for so, sw in subs:
                            go = o + so
                            nc.tensor.matmul(ps_ys[so][:, :sw], lhsT_ds[:],
                                             xs2[0:D, go:go + sw],
                                             start=False, stop=True,
                                             skip_group_check=True)
                    # drain sub-chunk y
                    for so, sw in subs:
                        go = o + so
                        if k == 1:
                            nc.scalar.copy(y_cm_acc[:, go:go + sw],
                                           ps_ys[so][:, :sw])
                        elif k == 3:
                            nc.vector.tensor_add(y_cm_acc[:, go:go + sw],
                                                 y_cm_acc[:, go:go + sw],
                                                 ps_ys[so][:, :sw])
                        elif k == 2:
                            nc.scalar.copy(y_mid[:, go:go + sw],
                                           ps_ys[so][:, :sw])
                        else:
                            nc.vector.tensor_add(y_mid[:, go:go + sw],
                                                 y_mid[:, go:go + sw],
                                                 ps_ys[so][:, :sw])
                    if k == 0:
                        # rm AR piece per forward chunk (k=2 already copied);
                        # these hide completely under the cm pair
                        alo, ahi = o, o + w
                        aw = ahi - alo
                        ari = dram.tile([D, aw], bf16, tag=f"ar_rm_in{ci}",
                                        name="ari", bufs=2)
                        aro = dram.tile([D, aw], bf16, tag=f"ar_rm_out{ci}",
                                        name="aro", bufs=2)
                        nc.sync.dma_start(ari[:], y_mid[:, alo:ahi])
                        nc.gpsimd.collective_compute(
                            "AllReduce", OP.add,
                            replica_groups=[[0, 1, 2, 3], [4, 5, 6, 7]],
                            ins=[ari[:].opt()], outs=[aro[:].opt()])
                        nc.sync.dma_start(y_mid[:, alo:ahi], aro[:])
                    if k == 3 and ci in (0, 1):
                        # cm AR pieces behind k=3's reverse sweep
                        alo = SCS if ci == 1 else 0
                        ahi = L if ci == 1 else SCS
                        aw = ahi - alo
                        ari = dram.tile([D, aw], bf16, tag=f"ar_cm_in{ci}",
                                        name="arci", bufs=2)
                        aro = dram.tile([D, aw], bf16, tag=f"ar_cm_out{ci}",
                                        name="arco", bufs=2)
                        nc.sync.dma_start(ari[:], y_cm_acc[:, alo:ahi])
                        nc.gpsimd.collective_compute(
                            "AllReduce", OP.add,
                            replica_groups=[[0, 1, 2, 3], [4, 5, 6, 7]],
                            ins=[ari[:].opt()], outs=[aro[:].opt()])
                        nc.sync.dma_start(y_cm_acc[:, alo:ahi], aro[:])
                if k == 3:
                    nc.vector.tensor_copy(
                        y_cm_g[:].rearrange("c (h w) -> c h w", h=48, w=48),
                        y_cm_acc[:].rearrange("c (w h) -> c h w", h=48, w=48))

            # ---- out_norm (batched smalls) + *silu(z) + out_proj + res ----
            y_h = stg.tile([D, L], bf16, tag="yh", name="y_h")

            ps_m5o = ps.tile([NCH, CS], f32, tag="sa", name="ps_m5o")
            ps_e5o = ps.tile([NCH, CS], f32, tag="sd", name="ps_e5o")
            nco = NCH - 1
            for j, (o, w) in enumerate(reversed(CHUNKS)):
                ci = nco - j
                nc.vector.tensor_add(y_h[:, o:o + w], y_mid[:, o:o + w],
                                     y_cm_g[:, o:o + w])
                ysq_c = sml.tile([D, CS], bf16, tag="ysq_c", name="ysq_c")
                nc.scalar.square(ysq_c[:, :w], y_h[:, o:o + w])
                nc.tensor.matmul(ps_m5o[:, :w], st64[:, 64 * ci:64 * ci + NCH],
                                 y_h[:, o:o + w],
                                 start=(j == 0), stop=(j == nco))
                nc.tensor.matmul(ps_e5o[:, :w], st64[:, 64 * ci:64 * ci + NCH],
                                 ysq_c[:, :w],
                                 start=(j == 0), stop=(j == nco))
            r5o, mr5o = ln_smalls5(ps_m5o, ps_e5o, "sm", bf16)

            prev_new = stg.tile([C_H, L], f32, tag="prev", name="prev_new")
            for j, (o, w) in enumerate(reversed(CHUNKS)):
                ci = nco - j
                ps_ra = ps.tile([D, CS], f32, tag="sa", name="ps_ra")
                nc.tensor.matmul(ps_ra[:, :w], selb[:, 64 * ci:64 * ci + D],
                                 r5o[:, :w], start=True, stop=True)
                ps_rb = ps.tile([D, CS], f32, tag="sd", name="ps_rb")
                nc.tensor.matmul(ps_rb[:, :w], selb[:, 64 * ci:64 * ci + D],
                                 mr5o[:, :w], start=True, stop=True)
                t1_c = sml.tile([D, CS], f32, tag="t1_c", name="t1_c")
                nc.vector.tensor_mul(t1_c[:, :w], y_h[:, o:o + w],
                                     ps_ra[:, :w])
                nc.vector.tensor_sub(t1_c[:, :w], t1_c[:, :w], ps_rb[:, :w])
                y2_c = sml.tile([D, CS], f32r, tag="y2_c", name="y2_c")
                nc.vector.tensor_mul(y2_c[:, :w], t1_c[:, :w],
                                     sz_sb[:, o:o + w])
                ps_op = ps.tile([C_H, CS], f32, tag="pd", name="ps_op", bufs=2)
                nc.tensor.matmul(ps_op[:, :w], lhsT_op[:],
                                 y2_c[:, :w], start=True, stop=True)
                nc.vector.scalar_tensor_tensor(
                    prev_new[:, o:o + w], s_t[:, o:o + w].bitcast(f32),
                    1.0 + vs, ps_op[:, :w], OP.mult, OP.add)
            prev_sb = prev_new
            nc.sync.dma_start(outs_cat[32 * i:32 * (i + 1), :], prev_new[:])

        # ---- final: x_res = cvm*x_shuf + outs_cat; LN over 128 ch ----
        nc.vector.scalar_tensor_tensor(xres[:],
                                       x_shuf[:].bitcast(f32), cvm,
                                       outs_cat[:], OP.mult, OP.add)
        out_sb = big.tile([C_IN, L], f32, tag="outs_cat", name="out_sb")
        xsq_l = big.tile([C_IN, L], f32r, tag="xxp2", name="xsq_l")
        ps_m5f = ps.tile([NCH, CS], f32, tag="sa", name="ps_m5f")
        ps_e5f = ps.tile([NCH, CS], f32, tag="sd", name="ps_e5f")
        for ci, (o, w) in enumerate(CHUNKS):
            nc.vector.tensor_mul(xsq_l[:, o:o + w],
                                 xres[:, o:o + w].bitcast(f32),
                                 xres[:, o:o + w].bitcast(f32))
            nc.tensor.matmul(ps_m5f[:, :w], st128[:, 64 * ci:64 * ci + NCH],
                             xres[:, o:o + w],
                             start=(ci == 0), stop=(ci == NCH - 1))
            nc.tensor.matmul(ps_e5f[:, :w], st128[:, 64 * ci:64 * ci + NCH],
                             xsq_l[:, o:o + w],
                             start=(ci == 0), stop=(ci == NCH - 1))
        r5f, mr5f = ln_smalls5(ps_m5f, ps_e5f, "sm", f32)
        for ci, (o, w) in enumerate(CHUNKS):
            ps_ra = ps.tile([C_IN, CS], f32, tag="sa", name="ps_ra3")
            nc.tensor.matmul(ps_ra[:, :w], sel128[:, 128 * ci:128 * ci + 128],
                             r5f[:, :w], start=True, stop=True)
            ps_rb = ps.tile([C_IN, CS], f32, tag="sd", name="ps_rb3")
            nc.tensor.matmul(ps_rb[:, :w], sel128[:, 128 * ci:128 * ci + 128],
                             mr5f[:, :w], start=True, stop=True)
            nc.vector.tensor_mul(out_sb[:, o:o + w],
                                 xres[:, o:o + w].bitcast(f32), ps_ra[:, :w])
            nc.vector.tensor_sub(out_sb[:, o:o + w], out_sb[:, o:o + w],
                                 ps_rb[:, :w])
            nc.vector.tensor_scalar(out_sb[:, o:o + w], out_sb[:, o:o + w],
                                    gamma[:], beta[:], OP.mult, OP.add)
            nc.sync.dma_start(out_d[:, o:o + w], out_sb[:, o:o + w])

    nc.compile()
    return nc


def _host_prep(inputs):
    import ml_dtypes
    bf16 = ml_dtypes.bfloat16
    x = np.asarray(inputs["x"], np.float32)
    ln1_w = np.asarray(inputs["ln1_w"], np.float32)
    ln1_b = np.asarray(inputs["ln1_b"], np.float32)
    in_proj_w = np.asarray(inputs["in_proj_w"], np.float32)
    conv_w = np.asarray(inputs["conv_w"], np.float32)
    conv_b = np.asarray(inputs["conv_b"], np.float32)
    x_proj_w = np.asarray(inputs["x_proj_w"], np.float32)
    dt_proj_w = np.asarray(inputs["dt_proj_w"], np.float32)
    dt_proj_b = np.asarray(inputs["dt_proj_b"], np.float32)
    A_logs = np.asarray(inputs["A_logs"], np.float32)
    Ds = np.asarray(inputs["Ds"], np.float32)
    out_norm_w = np.asarray(inputs["out_norm_w"], np.float32)
    out_norm_b = np.asarray(inputs["out_norm_b"], np.float32)
    out_proj_w = np.asarray(inputs["out_proj_w"], np.float32)
    final_ln_w = np.asarray(inputs["final_ln_w"], np.float32)
    final_ln_b = np.asarray(inputs["final_ln_b"], np.float32)
    assert not np.any(out_norm_b), "out_norm_b must be zero (folded)"

    # LN1 is applied to the input now: xx|z = W_ip @ (norm(s)*g + b);
    # gamma folds into W_ip, ln1_b goes through in_proj as a bias.
    W_ip = (in_proj_w * ln1_w[None, :]).astype(np.float32)        # (128, 32)
    bias_ip = (in_proj_w @ ln1_b).astype(np.float32)              # (128,)
    lhsT_ip = np.ascontiguousarray(W_ip.T)                        # (32, 128)
    w9_64 = np.ascontiguousarray(
        conv_w[:, :, 0, :].transpose(2, 0, 1).reshape(D, 9))      # (64, 9)
    w9 = np.concatenate([w9_64, w9_64], 0)                        # (128, 9)
    convb2 = np.concatenate([conv_b, conv_b]).reshape(C_IN, 1)
    A = -np.exp(A_logs)                                           # (K, 64, 16)
    Ds_q = (Ds.sum(0) / 4.0).astype(np.float32)                   # (64,)
    lhsT_ds = np.zeros((D, D), bf16)
    for d in range(D):
        lhsT_ds[d, d] = Ds_q[d]
    W_op = (out_proj_w * out_norm_w[None, :]).astype(np.float32)  # (32, 64)
    lhsT_op = np.ascontiguousarray(W_op.T)                        # (64, 32)

    M = np.einsum("kdr,krc->kdc", dt_proj_w, x_proj_w[:, :DT_RANK, :])
    lhsT_M2 = np.zeros((D, K, C_IN), bf16)
    for k in range(K):
        lhsT_M2[:, k, 0:D] = M[k].T
        lhsT_M2[:, k, D:C_IN] = M[k].T
    dtb2 = np.zeros((C_IN, K), np.float32)
    dtb2[0:D] = dt_proj_b.T
    dtb2[D:C_IN] = dt_proj_b.T

    # select-row / stats lhsT blocks (64-aligned per chunk)
    sel64 = np.zeros((NCH, NCH * 64), np.float32)
    sel128 = np.zeros((NCH, NCH * 128), np.float32)
    for ci in range(NCH):
        sel64[ci, 64 * ci:64 * ci + 64] = 1.0
        sel128[ci, 128 * ci:128 * ci + 128] = 1.0
    st32 = np.zeros((C_H, NCH * 64), np.float32)
    st64 = np.zeros((D, NCH * 64), np.float32)
    st128 = np.zeros((C_IN, NCH * 64), np.float32)
    for ci in range(NCH):
        st32[:, 64 * ci + ci] = 1.0 / C_H
        st64[:, 64 * ci + ci] = 1.0 / D
        st128[:, 64 * ci + ci] = 1.0 / C_IN
    lhsT_ys = np.zeros((C_IN, D), bf16)
    for p in range(C_IN):
        lhsT_ys[p, p % D] = 1.0

    common = {
        "lhsT_ip": lhsT_ip,
        "bias_xx": bias_ip[0:D].reshape(D, 1),
        "bias_z": bias_ip[D:C_IN].reshape(D, 1),
        "w9": w9, "convb2": convb2,
        "sel128": sel128, "selb": sel64.astype(bf16),
        "st32": st32, "st64": st64.astype(bf16), "st128": st128,
        "lhsT_M2": lhsT_M2, "dtb2": dtb2,
        "lhsT_ys": lhsT_ys, "lhsT_ds": lhsT_ds,
        "lhsT_op": lhsT_op,
        "gamma": final_ln_w.reshape(C_IN, 1),
        "beta": final_ln_b.reshape(C_IN, 1),
    }

    g = HEAD
    per_b = []
    for b in range(B):
        xs = x[b].reshape(H, W, g, C_IN // g).transpose(0, 1, 3, 2).reshape(L, C_IN)
        per_b.append(np.ascontiguousarray(xs.T))

    in_maps = []
    for c in range(NCORES):
        b, nh = c // 4, c % 4
        A2 = np.zeros((C_IN, K, 2), np.float32)
        lhsT_B = np.zeros((D, K, 2, C_IN), bf16)
        lhsT_C = np.zeros((D, K, 2, C_IN), bf16)
        for k in range(K):
            for gl in range(2):
                for half in range(2):
                    n = 4 * nh + 2 * gl + half
                    rows = slice(64 * half, 64 * half + 64)
                    A2[rows, k, gl] = A[k, :, n]
                    lhsT_B[:, k, gl, rows] = x_proj_w[k, DT_RANK + n, :][:, None]
                    lhsT_C[:, k, gl, rows] = x_proj_w[k, DT_RANK + N + n, :][:, None]
        in_maps.append(dict(common, x_shuf=per_b[b],
                            A2=A2, lhsT_B=lhsT_B, lhsT_C=lhsT_C))
    vs = float(np.asarray(inputs["vss_skip"]).ravel()[0])
    cvm = float(np.asarray(inputs["cvm_skip"]).ravel()[0])
    return in_maps, vs, cvm


def kernel(**inputs) -> np.ndarray:
    from concourse.bass_utils import run_bass_kernel_spmd

    in_maps, vs, cvm = _host_prep(inputs)
    key = (vs, cvm)
    if key not in _cache:
        _cache[key] = _build(vs, cvm)
    nc = _cache[key]
    res = run_bass_kernel_spmd(nc, in_maps, core_ids=list(range(NCORES)))
    out = np.zeros((B, H, W, C_IN), np.float32)
    for b in range(B):
        out_cf = res.results[4 * b]["out_cf"]
        out[b] = out_cf.T.reshape(H, W, C_IN)
    return out
